# revision 10
# baseline (speedup 1.0000x reference)
"""PixelmIoU Trainium2 kernel.

Strategy
--------
The reference computes per-pixel argmax over 21 class logits, then three
21-bin histograms (pred / gt / intersection counts) and finally mIoU.
The only heavy part is streaming the 352 MB `pred` tensor and reducing
21 planes to a per-pixel argmax — everything downstream touches 4.2M
small ints and is host-trivial.

Device (8 cores, batch-sharded, 2 batches/core):
  * Each class plane chunk is DMA'd to SBUF ([128, NFREE] f32 tiles).
  * GpSimd "encodes" each chunk in place: clear the low 5 mantissa bits
    and OR in a 5-bit class code (31 - c).  Encoding is order-preserving
    up to 2^-18 relative quantization, so a plain f32 running MAX over
    the 21 encoded planes (on DVE, 1x ports, fully parallel with GpSimd)
    yields the max value *and* its argmax in the low 5 bits.  Ties after
    quantization resolve to the smallest class index for positive maxima
    (the common case), matching jnp.argmax.
  * DVE extracts the 5-bit code to uint8 and DMA's it out (0.5 MB/core).

Host: bincounts of pm/gt/intersection + the mIoU formula.

Engine budget per core: DMA ~44 MB ≈ 123 us (the roofline), GpSimd
encodes ~75 us, DVE max-chain ~85 us — compute hides under the stream.
"""

import os
import numpy as np

B, C, H, W = 16, 21, 512, 512
N_CORES = 8
B_PER = B // N_CORES            # batches per core
PLANE = H * W                   # 262144 px per plane
NFREE = 2048                    # tile free-dim (pixels)
G_PER_BATCH = PLANE // (128 * NFREE)   # pixel groups per batch (=2)

ENC_MASK = 0xFFFFFFE0           # clear low 5 mantissa bits
CODE_BITS = 31

_built = {}


def _import_concourse():
    try:
        import concourse.bass  # noqa: F401
    except ImportError:
        import sys
        for p in ("/opt/trn_rl_repo", "/root/.axon_site/_ro/trn_rl_repo"):
            if os.path.isdir(p) and p not in sys.path:
                sys.path.append(p)


def _build():
    """Build + compile the per-core Bass program (SPMD, identical on all cores)."""
    if "nc" in _built:
        return _built["nc"]
    _import_concourse()
    from contextlib import ExitStack
    import concourse.tile as tile
    from concourse import bacc, mybir

    nc = bacc.Bacc(
        "TRN2",
        target_bir_lowering=False,
        debug=False,
        enable_asserts=True,
        num_devices=N_CORES,
    )
    pred = nc.dram_tensor(
        "pred", [B_PER, C, H, W], mybir.dt.float32, kind="ExternalInput"
    ).ap()
    code_out = nc.dram_tensor(
        "code", [B_PER, H, W], mybir.dt.uint32, kind="ExternalOutput"
    ).ap()

    # [b, c, g, p, n]: group-major, 128 partitions of NFREE contiguous pixels
    pred_t = pred.rearrange("b c h w -> b c (h w)").rearrange(
        "b c (g p n) -> b c g p n", p=128, n=NFREE
    )
    out_t = code_out.rearrange("b h w -> b (h w)").rearrange(
        "b (g p n) -> b g p n", p=128, n=NFREE
    )

    with tile.TileContext(nc) as tc, ExitStack() as ctx:
        planes = ctx.enter_context(tc.tile_pool(name="planes", bufs=8))
        accs = ctx.enter_context(tc.tile_pool(name="accs", bufs=3))

        for b in range(B_PER):
            for g in range(G_PER_BATCH):
                acc = accs.tile([128, NFREE], mybir.dt.float32)
                first = None
                for c in range(C):
                    t = planes.tile([128, NFREE], mybir.dt.float32)
                    nc.sync.dma_start(t[:], pred_t[b, c, g])
                    tu = t[:].bitcast(mybir.dt.uint32)
                    # encode in place: (bits & ~31) | (31 - c)
                    nc.vector.tensor_scalar(
                        tu,
                        tu,
                        ENC_MASK,
                        CODE_BITS - c,
                        op0=mybir.AluOpType.bitwise_and,
                        op1=mybir.AluOpType.bitwise_or,
                    )
                    if c == 0:
                        first = t
                    elif c == 1:
                        nc.vector.tensor_max(acc[:], first[:], t[:])
                    else:
                        nc.vector.tensor_max(acc[:], acc[:], t[:])
                # ship the raw encoded max bits; host extracts the 5-bit code
                nc.gpsimd.dma_start(out_t[b, g], acc[:].bitcast(mybir.dt.uint32))

    nc.compile()
    _built["nc"] = nc
    return nc


def _run_device(pred: np.ndarray, trace: bool = False):
    """Run the SPMD kernel; returns (pm [16,512,512] int64, BassKernelResults)."""
    _import_concourse()
    from concourse.bass_utils import run_bass_kernel_spmd

    nc = _build()
    in_maps = [
        {"pred": np.ascontiguousarray(pred[B_PER * i : B_PER * (i + 1)])}
        for i in range(N_CORES)
    ]
    res = run_bass_kernel_spmd(nc, in_maps, list(range(N_CORES)), trace=trace)
    enc = np.concatenate([res.results[i]["code"] for i in range(N_CORES)], axis=0)
    pm = CODE_BITS - (enc.astype(np.int64) & CODE_BITS)
    return pm, res


def _miou_from_pm(pm: np.ndarray, gt: np.ndarray) -> np.float32:
    gtf = gt.reshape(-1).astype(np.int64)
    pmf = pm.reshape(-1)
    match = pmf == gtf
    inter = np.bincount(pmf[match], minlength=C)[:C].astype(np.float64)
    cp = np.bincount(pmf, minlength=C)[:C].astype(np.float64)
    cg = np.bincount(gtf, minlength=C)[:C].astype(np.float64)
    union = cp + cg - inter
    iou = np.where(inter > 0, inter / np.where(union > 0, union, 1.0), 0.0)
    return np.float32(iou.mean())


def kernel(pred: np.ndarray, gt: np.ndarray) -> np.ndarray:
    pred = np.asarray(pred, dtype=np.float32)
    gt = np.asarray(gt, dtype=np.int32)
    pm, _ = _run_device(pred)
    return np.asarray(_miou_from_pm(pm, gt))
